# revision 7
# baseline (speedup 1.0000x reference)
"""Trainium2 Bass kernel for per-sample multi-head Linear (MoE-style routing).

Computes logits[i] = x[i] @ W[system_id[i]].T + b[system_id[i]] for
x:[B,D]=[262144,256], W:[S,C,D]=[16,10,256], b:[S,C], int system ids.

Strategy: data-parallel over 8 NeuronCores (32768 rows each), with the
per-row head selection folded into the matmul itself ("select-via-max"):

  ps[b, (c,s)] = x[b] @ Wt[:, (c,s)] + onehot[b] @ V[:, (c,s)]
  where V[k, (c,s)] = b[k,c] if s == k else -1e30

so every lane belonging to a head other than the row's own sits at ~-1e30
and the row's own lane holds the exact fp32 logit + bias. The selection is
then a single segmented reduce_max over the 16 systems -- no per-row mask
multiply, no separate bias add.

Per core, per 4096-row x-tile (bf16 throughout -> half the HBM traffic):
  - 3 matmuls per 128-row subtile (two k=128 halves of x, plus the onehot
    "penalty" matmul whose stationary is zero-padded to K=128 on device:
    mixing K=16 and K=128 stationaries stalls the PE pipeline ~3x),
  - PSUM packs 2 subtiles per bank [128, 320]; copies to SBUF alternate
    between the Scalar and Vector engines,
  - one reduce_max per 8 subtiles, output DMA issued from GpSimd,
  - a short PE warmup burst covers the first DMA ramp (HAM un-throttle).
"""

import sys
import numpy as np

if "/opt/trn_rl_repo" not in sys.path:
    sys.path.insert(0, "/opt/trn_rl_repo")

import concourse.bacc as bacc
import concourse.bass as bass
import concourse.mybir as mybir
import concourse.tile as tile
from concourse.bass_utils import run_bass_kernel_spmd

B = 262144
D = 256
S = 16
C = 10
N_CORES = 8
B_CORE = B // N_CORES  # 32768

SC = S * C   # 160
SUB_B = 128  # rows per matmul subtile

F32 = mybir.dt.float32
BF16 = mybir.dt.bfloat16


def build_nc(
    n_rows: int = B_CORE,
    dma_b: int = 4096,
    psum_bufs: int = 8,
    xt_bufs: int = 3,
    megap_bufs: int = 3,
    oh_bufs_n: int = 3,
    red_subs: int = 8,
    warmup_mms: int = 30,
):
    """Build the per-core Bass program. Same program runs SPMD on all cores."""
    assert n_rows % dma_b == 0
    n_dma = n_rows // dma_b
    subs_per_dma = dma_b // SUB_B
    packs_per_dma = subs_per_dma // 2
    assert subs_per_dma % red_subs == 0 and red_subs % 2 == 0

    nc = bacc.Bacc(
        "TRN2",
        target_bir_lowering=False,
        debug=False,
        num_devices=N_CORES,
    )

    xT = nc.dram_tensor("xT", [D, n_rows], BF16, kind="ExternalInput")
    oh = nc.dram_tensor("oh", [S, n_rows], BF16, kind="ExternalInput")
    # wt[d, c*S + s] = W[s, c, d]  (class-major, systems innermost)
    wt = nc.dram_tensor("wt", [D, SC], BF16, kind="ExternalInput")
    # vpen[k, c*S + s] = b[k, c] if s == k else -1e30
    vpen = nc.dram_tensor("vpen", [S, SC], BF16, kind="ExternalInput")
    out = nc.dram_tensor("out", [n_rows, C], F32, kind="ExternalOutput")

    with tile.TileContext(nc) as tc:
        with (
            tc.tile_pool(name="consts", bufs=1) as consts,
            tc.tile_pool(name="xtp0", bufs=xt_bufs) as xtp0,
            tc.tile_pool(name="xtp1", bufs=xt_bufs) as xtp1,
            tc.tile_pool(name="megap", bufs=megap_bufs) as megap,
            tc.tile_pool(name="outp", bufs=4) as outp,
            tc.tile_pool(name="psum", bufs=psum_bufs, space=bass.MemorySpace.PSUM) as psump,
        ):
            wt0 = consts.tile([SUB_B, SC], BF16, tag="wt0")
            wt1 = consts.tile([SUB_B, SC], BF16, tag="wt1")
            vpen_t = consts.tile([SUB_B, SC], BF16, tag="vpen")
            nc.sync.dma_start(wt0[:], wt[0:SUB_B, :])
            nc.sync.dma_start(wt1[:], wt[SUB_B : 2 * SUB_B, :])
            # vpen zero-padded to 128 partitions; rows 16..127 stay zero.
            nc.gpsimd.memset(vpen_t[:], 0)
            nc.sync.dma_start(vpen_t[0:S, :], vpen[:])

            # Manually-cycled zero-padded onehot buffers (rows 16..127 stay 0)
            # so every matmul stationary is a uniform [128, 128] tile.
            oh_ts = []
            for i in range(oh_bufs_n):
                t = consts.tile(
                    [SUB_B, dma_b], BF16, tag=f"ohpad{i}", name=f"ohpad{i}"
                )
                [nc.vector, nc.gpsimd, nc.vector][i % 3].memset(t[:], 0)
                oh_ts.append(t)

            # Warmup burst: keeps the PE busy through the first DMA ramp and
            # brings HAM to full clock before the real stream starts.
            wps = psump.tile([SUB_B, 2 * SC], F32, tag="ps", name="wps")
            for _ in range(warmup_mms):
                nc.tensor.matmul(
                    wps[:, 0:SC], wt0[:, 0:SUB_B], wt1[:], start=True, stop=True
                )

            out_r = out.rearrange("(n j p) c -> n p j c", p=SUB_B, j=subs_per_dma)

            for di in range(n_dma):
                xt0 = xtp0.tile([SUB_B, dma_b], BF16, tag="xt0")
                xt1 = xtp1.tile([SUB_B, dma_b], BF16, tag="xt1")
                oh_t = oh_ts[di % oh_bufs_n]
                c0 = di * dma_b
                nc.sync.dma_start(xt0[:], xT[0:SUB_B, c0 : c0 + dma_b])
                nc.scalar.dma_start(xt1[:], xT[SUB_B : 2 * SUB_B, c0 : c0 + dma_b])
                nc.sync.dma_start(oh_t[0:S, :], oh[:, c0 : c0 + dma_b])

                outb = outp.tile([SUB_B, subs_per_dma * C], F32, tag="outb")
                mega = megap.tile([SUB_B, subs_per_dma * SC], BF16, tag="mega")

                packs_per_red = red_subs // 2
                for pk in range(packs_per_dma):
                    ps = psump.tile([SUB_B, 2 * SC], F32, tag="ps", name="ps")
                    for h in range(2):
                        jj = pk * 2 + h
                        js = jj * SUB_B
                        lo, hi = h * SC, (h + 1) * SC
                        nc.tensor.matmul(
                            ps[:, lo:hi], xt0[:, js : js + SUB_B], wt0[:],
                            start=True, stop=False,
                        )
                        nc.tensor.matmul(
                            ps[:, lo:hi], xt1[:, js : js + SUB_B], wt1[:],
                            start=False, stop=False,
                        )
                        nc.tensor.matmul(
                            ps[:, lo:hi], oh_t[:, js : js + SUB_B], vpen_t[:],
                            start=False, stop=True,
                        )
                    if pk % 2 == 1:
                        nc.vector.tensor_copy(
                            mega[:, pk * 2 * SC : (pk + 1) * 2 * SC], ps[:]
                        )
                    else:
                        nc.scalar.copy(
                            mega[:, pk * 2 * SC : (pk + 1) * 2 * SC], ps[:]
                        )

                    if (pk + 1) % packs_per_red == 0:
                        r = pk // packs_per_red
                        m0 = r * red_subs * C
                        m1 = (r + 1) * red_subs * C
                        nc.vector.tensor_reduce(
                            out=outb[:, m0:m1],
                            in_=mega[:, m0 * S : m1 * S].rearrange(
                                "p (m s) -> p m s", m=m1 - m0, s=S
                            ),
                            axis=mybir.AxisListType.X,
                            op=mybir.AluOpType.max,
                        )
                nc.gpsimd.dma_start(
                    out_r[di],
                    outb[:].rearrange("p (j c) -> p j c", j=subs_per_dma, c=C),
                )

    nc.compile()
    return nc


def _round_bf16(a: np.ndarray) -> np.ndarray:
    """fp32 -> bf16 with round-to-nearest-even, returned as ml_dtypes.bfloat16."""
    import ml_dtypes

    bits = np.ascontiguousarray(a, dtype=np.float32).view(np.uint32)
    lsb = (bits >> np.uint32(16)) & np.uint32(1)
    rounded = ((bits + np.uint32(0x7FFF) + lsb) >> np.uint32(16)).astype(np.uint16)
    return rounded.view(ml_dtypes.bfloat16)


def _host_prep(W, b):
    """Weight-stack layout prep shared by all cores."""
    W = np.asarray(W, dtype=np.float32)
    b = np.asarray(b, dtype=np.float32)
    wt = _round_bf16(np.transpose(W, (2, 1, 0)).reshape(D, SC))
    vpen = np.full((S, SC), -1e30, dtype=np.float32)
    for k in range(S):
        vpen[k, np.arange(C) * S + k] = b[k]
    vpen = _round_bf16(vpen)
    return wt, vpen


_NC_CACHE = {}


def kernel(x, system_id, W, b):
    x = np.asarray(x, dtype=np.float32)
    system_id = np.asarray(system_id)

    key = (x.shape[0],)
    if key not in _NC_CACHE:
        _NC_CACHE[key] = build_nc(x.shape[0] // N_CORES)
    nc = _NC_CACHE[key]

    wt, vpen = _host_prep(W, b)

    n_rows = x.shape[0] // N_CORES
    x_bf = _round_bf16(x)
    eye = np.eye(S, dtype=np.float32)
    in_maps = []
    for core in range(N_CORES):
        lo, hi = core * n_rows, (core + 1) * n_rows
        in_maps.append(
            {
                "xT": np.ascontiguousarray(x_bf[lo:hi].T),           # [D, n_rows]
                "oh": np.ascontiguousarray(
                    _round_bf16(eye[:, system_id[lo:hi]])
                ),                                                    # [S, n_rows]
                "wt": wt,
                "vpen": vpen,
            }
        )

    res = run_bass_kernel_spmd(nc, in_maps, core_ids=list(range(N_CORES)))
    out = np.concatenate([res.results[i]["out"] for i in range(N_CORES)], axis=0)
    return out.astype(np.float32)


# revision 8
# speedup vs baseline: 1.0424x; 1.0424x over previous
"""Trainium2 Bass kernel for per-sample multi-head Linear (MoE-style routing).

Computes logits[i] = x[i] @ W[system_id[i]].T + b[system_id[i]] for
x:[B,D]=[262144,256], W:[S,C,D]=[16,10,256], b:[S,C], int system ids.

Strategy: data-parallel over 8 NeuronCores (32768 rows each), with the
per-row head selection folded into the matmul itself ("select-via-max"):

  ps[b, (c,s)] = x[b] @ Wt[:, (c,s)] + onehot[b] @ V[:, (c,s)]
  where V[k, (c,s)] = b[k,c] if s == k else -1e30

so every lane belonging to a head other than the row's own sits at ~-1e30
and the row's own lane holds the exact fp32 logit + bias. The selection is
then a single segmented reduce_max over the 16 systems -- no per-row mask
multiply, no separate bias add.

Per core, per 2048-row x-tile (bf16 throughout -> half the HBM traffic):
  - 3 matmuls per 128-row subtile (two k=128 halves of x, plus the onehot
    "penalty" matmul whose stationary is zero-padded to K=128 on device:
    mixing K=16 and K=128 stationaries stalls the PE pipeline ~3x),
  - PSUM packs 2 subtiles per bank [128, 320]; copies to SBUF alternate
    between the Scalar and Vector engines,
  - one reduce_max per 8 subtiles, output DMA issued from GpSimd,
  - a short PE warmup burst covers the first DMA ramp (HAM un-throttle).
"""

import sys
import numpy as np

if "/opt/trn_rl_repo" not in sys.path:
    sys.path.insert(0, "/opt/trn_rl_repo")

import concourse.bacc as bacc
import concourse.bass as bass
import concourse.mybir as mybir
import concourse.tile as tile
from concourse.bass_utils import run_bass_kernel_spmd

B = 262144
D = 256
S = 16
C = 10
N_CORES = 8
B_CORE = B // N_CORES  # 32768

SC = S * C   # 160
SUB_B = 128  # rows per matmul subtile

F32 = mybir.dt.float32
BF16 = mybir.dt.bfloat16


def build_nc(
    n_rows: int = B_CORE,
    dma_b: int = 2048,
    psum_bufs: int = 8,
    xt_bufs: int = 3,
    megap_bufs: int = 4,
    oh_bufs_n: int = 3,
    red_subs: int = 8,
    warmup_mms: int = 30,
):
    """Build the per-core Bass program. Same program runs SPMD on all cores."""
    assert n_rows % dma_b == 0
    n_dma = n_rows // dma_b
    subs_per_dma = dma_b // SUB_B
    packs_per_dma = subs_per_dma // 2
    assert subs_per_dma % red_subs == 0 and red_subs % 2 == 0

    nc = bacc.Bacc(
        "TRN2",
        target_bir_lowering=False,
        debug=False,
        num_devices=N_CORES,
    )

    xT = nc.dram_tensor("xT", [D, n_rows], BF16, kind="ExternalInput")
    oh = nc.dram_tensor("oh", [S, n_rows], BF16, kind="ExternalInput")
    # wt[d, c*S + s] = W[s, c, d]  (class-major, systems innermost)
    wt = nc.dram_tensor("wt", [D, SC], BF16, kind="ExternalInput")
    # vpen[k, c*S + s] = b[k, c] if s == k else -1e30
    vpen = nc.dram_tensor("vpen", [S, SC], BF16, kind="ExternalInput")
    out = nc.dram_tensor("out", [n_rows, C], F32, kind="ExternalOutput")

    with tile.TileContext(nc) as tc:
        with (
            tc.tile_pool(name="consts", bufs=1) as consts,
            tc.tile_pool(name="xtp0", bufs=xt_bufs) as xtp0,
            tc.tile_pool(name="xtp1", bufs=xt_bufs) as xtp1,
            tc.tile_pool(name="megap", bufs=megap_bufs) as megap,
            tc.tile_pool(name="outp", bufs=4) as outp,
            tc.tile_pool(name="psum", bufs=psum_bufs, space=bass.MemorySpace.PSUM) as psump,
        ):
            wt0 = consts.tile([SUB_B, SC], BF16, tag="wt0")
            wt1 = consts.tile([SUB_B, SC], BF16, tag="wt1")
            vpen_t = consts.tile([SUB_B, SC], BF16, tag="vpen")
            nc.sync.dma_start(wt0[:], wt[0:SUB_B, :])
            nc.sync.dma_start(wt1[:], wt[SUB_B : 2 * SUB_B, :])
            # vpen zero-padded to 128 partitions; rows 16..127 stay zero.
            nc.gpsimd.memset(vpen_t[:], 0)
            nc.sync.dma_start(vpen_t[0:S, :], vpen[:])

            # Manually-cycled zero-padded onehot buffers (rows 16..127 stay 0)
            # so every matmul stationary is a uniform [128, 128] tile.
            oh_ts = []
            for i in range(oh_bufs_n):
                t = consts.tile(
                    [SUB_B, dma_b], BF16, tag=f"ohpad{i}", name=f"ohpad{i}"
                )
                [nc.vector, nc.gpsimd, nc.vector][i % 3].memset(t[:], 0)
                oh_ts.append(t)

            # Warmup burst: keeps the PE busy through the first DMA ramp and
            # brings HAM to full clock before the real stream starts.
            wps = psump.tile([SUB_B, 2 * SC], F32, tag="ps", name="wps")
            for _ in range(warmup_mms):
                nc.tensor.matmul(
                    wps[:, 0:SC], wt0[:, 0:SUB_B], wt1[:], start=True, stop=True
                )

            out_r = out.rearrange("(n j p) c -> n p j c", p=SUB_B, j=subs_per_dma)

            for di in range(n_dma):
                xt0 = xtp0.tile([SUB_B, dma_b], BF16, tag="xt0")
                xt1 = xtp1.tile([SUB_B, dma_b], BF16, tag="xt1")
                oh_t = oh_ts[di % oh_bufs_n]
                c0 = di * dma_b
                nc.sync.dma_start(xt0[:], xT[0:SUB_B, c0 : c0 + dma_b])
                nc.sync.dma_start(xt1[:], xT[SUB_B : 2 * SUB_B, c0 : c0 + dma_b])
                nc.scalar.dma_start(oh_t[0:S, :], oh[:, c0 : c0 + dma_b])

                outb = outp.tile([SUB_B, subs_per_dma * C], F32, tag="outb")
                mega = megap.tile([SUB_B, subs_per_dma * SC], BF16, tag="mega")

                packs_per_red = red_subs // 2
                for pk in range(packs_per_dma):
                    ps = psump.tile([SUB_B, 2 * SC], F32, tag="ps", name="ps")
                    for h in range(2):
                        jj = pk * 2 + h
                        js = jj * SUB_B
                        lo, hi = h * SC, (h + 1) * SC
                        nc.tensor.matmul(
                            ps[:, lo:hi], xt0[:, js : js + SUB_B], wt0[:],
                            start=True, stop=False,
                        )
                        nc.tensor.matmul(
                            ps[:, lo:hi], xt1[:, js : js + SUB_B], wt1[:],
                            start=False, stop=False,
                        )
                        nc.tensor.matmul(
                            ps[:, lo:hi], oh_t[:, js : js + SUB_B], vpen_t[:],
                            start=False, stop=True,
                        )
                    if pk % 2 == 1:
                        nc.vector.tensor_copy(
                            mega[:, pk * 2 * SC : (pk + 1) * 2 * SC], ps[:]
                        )
                    else:
                        nc.scalar.copy(
                            mega[:, pk * 2 * SC : (pk + 1) * 2 * SC], ps[:]
                        )

                    if (pk + 1) % packs_per_red == 0:
                        r = pk // packs_per_red
                        m0 = r * red_subs * C
                        m1 = (r + 1) * red_subs * C
                        nc.vector.tensor_reduce(
                            out=outb[:, m0:m1],
                            in_=mega[:, m0 * S : m1 * S].rearrange(
                                "p (m s) -> p m s", m=m1 - m0, s=S
                            ),
                            axis=mybir.AxisListType.X,
                            op=mybir.AluOpType.max,
                        )
                nc.gpsimd.dma_start(
                    out_r[di],
                    outb[:].rearrange("p (j c) -> p j c", j=subs_per_dma, c=C),
                )

    nc.compile()
    return nc


def _round_bf16(a: np.ndarray) -> np.ndarray:
    """fp32 -> bf16 with round-to-nearest-even, returned as ml_dtypes.bfloat16."""
    import ml_dtypes

    bits = np.ascontiguousarray(a, dtype=np.float32).view(np.uint32)
    lsb = (bits >> np.uint32(16)) & np.uint32(1)
    rounded = ((bits + np.uint32(0x7FFF) + lsb) >> np.uint32(16)).astype(np.uint16)
    return rounded.view(ml_dtypes.bfloat16)


def _host_prep(W, b):
    """Weight-stack layout prep shared by all cores."""
    W = np.asarray(W, dtype=np.float32)
    b = np.asarray(b, dtype=np.float32)
    wt = _round_bf16(np.transpose(W, (2, 1, 0)).reshape(D, SC))
    vpen = np.full((S, SC), -1e30, dtype=np.float32)
    for k in range(S):
        vpen[k, np.arange(C) * S + k] = b[k]
    vpen = _round_bf16(vpen)
    return wt, vpen


_NC_CACHE = {}


def kernel(x, system_id, W, b):
    x = np.asarray(x, dtype=np.float32)
    system_id = np.asarray(system_id)

    key = (x.shape[0],)
    if key not in _NC_CACHE:
        _NC_CACHE[key] = build_nc(x.shape[0] // N_CORES)
    nc = _NC_CACHE[key]

    wt, vpen = _host_prep(W, b)

    n_rows = x.shape[0] // N_CORES
    x_bf = _round_bf16(x)
    eye = np.eye(S, dtype=np.float32)
    in_maps = []
    for core in range(N_CORES):
        lo, hi = core * n_rows, (core + 1) * n_rows
        in_maps.append(
            {
                "xT": np.ascontiguousarray(x_bf[lo:hi].T),           # [D, n_rows]
                "oh": np.ascontiguousarray(
                    _round_bf16(eye[:, system_id[lo:hi]])
                ),                                                    # [S, n_rows]
                "wt": wt,
                "vpen": vpen,
            }
        )

    res = run_bass_kernel_spmd(nc, in_maps, core_ids=list(range(N_CORES)))
    out = np.concatenate([res.results[i]["out"] for i in range(N_CORES)], axis=0)
    return out.astype(np.float32)


# revision 9
# speedup vs baseline: 1.1390x; 1.0927x over previous
"""Trainium2 Bass kernel for per-sample multi-head Linear (MoE-style routing).

Computes logits[i] = x[i] @ W[system_id[i]].T + b[system_id[i]] for
x:[B,D]=[262144,256], W:[S,C,D]=[16,10,256], b:[S,C], int system ids.

Strategy: data-parallel over 8 NeuronCores (32768 rows each), with the
per-row head selection folded into the matmul itself ("select-via-max"):

  ps[b, (c,s)] = x[b] @ Wt[:, (c,s)] + onehot[b] @ V[:, (c,s)]
  where V[k, (c,s)] = b[k,c] if s == k else -1e30

so every lane belonging to a head other than the row's own sits at ~-1e30
and the row's own lane holds the exact fp32 logit + bias. The selection is
then a single segmented reduce_max over the 16 systems -- no per-row mask
multiply, no separate bias add.

Per core, per 2048-row x-tile (bf16 throughout -> half the HBM traffic):
  - 3 matmuls per 128-row subtile (two k=128 halves of x, plus the onehot
    "penalty" matmul whose stationary is zero-padded to K=128 on device:
    mixing K=16 and K=128 stationaries stalls the PE pipeline ~3x),
  - PSUM packs 2 subtiles per bank [128, 320]; copies to SBUF alternate
    between the Scalar and Vector engines,
  - one reduce_max per 8 subtiles, output DMA issued from GpSimd,
  - a short PE warmup burst covers the first DMA ramp (HAM un-throttle).
"""

import sys
import numpy as np

if "/opt/trn_rl_repo" not in sys.path:
    sys.path.insert(0, "/opt/trn_rl_repo")

import concourse.bacc as bacc
import concourse.bass as bass
import concourse.mybir as mybir
import concourse.tile as tile
from concourse.bass_utils import run_bass_kernel_spmd

B = 262144
D = 256
S = 16
C = 10
N_CORES = 8
B_CORE = B // N_CORES  # 32768

SC = S * C   # 160
SUB_B = 128  # rows per matmul subtile

F32 = mybir.dt.float32
BF16 = mybir.dt.bfloat16


def build_nc(
    n_rows: int = B_CORE,
    dma_b: int = 2048,
    psum_bufs: int = 8,
    xt_bufs: int = 3,
    megap_bufs: int = 4,
    oh_bufs_n: int = 3,
    red_subs: int = 8,
    warmup_mms: int = 30,
):
    """Build the per-core Bass program. Same program runs SPMD on all cores."""
    assert n_rows % dma_b == 0
    n_dma = n_rows // dma_b
    subs_per_dma = dma_b // SUB_B
    packs_per_dma = subs_per_dma // 2
    assert subs_per_dma % red_subs == 0 and red_subs % 2 == 0

    nc = bacc.Bacc(
        "TRN2",
        target_bir_lowering=False,
        debug=False,
        num_devices=N_CORES,
    )

    xT = nc.dram_tensor("xT", [D, n_rows], BF16, kind="ExternalInput")
    oh = nc.dram_tensor("oh", [S, n_rows], BF16, kind="ExternalInput")
    # wt[d, c*S + s] = W[s, c, d]  (class-major, systems innermost)
    wt = nc.dram_tensor("wt", [D, SC], BF16, kind="ExternalInput")
    # vpen[k, c*S + s] = b[k, c] if s == k else -1e30
    vpen = nc.dram_tensor("vpen", [S, SC], BF16, kind="ExternalInput")
    out = nc.dram_tensor("out", [n_rows, C], F32, kind="ExternalOutput")

    with tile.TileContext(nc) as tc:
        with (
            tc.tile_pool(name="consts", bufs=1) as consts,
            tc.tile_pool(name="xtp0", bufs=xt_bufs) as xtp0,
            tc.tile_pool(name="xtp1", bufs=xt_bufs) as xtp1,
            tc.tile_pool(name="megap", bufs=megap_bufs) as megap,
            tc.tile_pool(name="outp", bufs=4) as outp,
            tc.tile_pool(name="psum", bufs=psum_bufs, space=bass.MemorySpace.PSUM) as psump,
        ):
            wt0 = consts.tile([SUB_B, SC], BF16, tag="wt0")
            wt1 = consts.tile([SUB_B, SC], BF16, tag="wt1")
            vpen_t = consts.tile([SUB_B, SC], BF16, tag="vpen")
            nc.sync.dma_start(wt0[:], wt[0:SUB_B, :])
            nc.sync.dma_start(wt1[:], wt[SUB_B : 2 * SUB_B, :])
            # vpen zero-padded to 128 partitions; rows 16..127 stay zero.
            nc.gpsimd.memset(vpen_t[:], 0)
            nc.sync.dma_start(vpen_t[0:S, :], vpen[:])

            # Manually-cycled zero-padded onehot buffers (rows 16..127 stay 0)
            # so every matmul stationary is a uniform [128, 128] tile.
            oh_ts = []
            for i in range(oh_bufs_n):
                t = consts.tile(
                    [SUB_B, dma_b], BF16, tag=f"ohpad{i}", name=f"ohpad{i}"
                )
                [nc.vector, nc.gpsimd, nc.vector][i % 3].memset(t[:], 0)
                oh_ts.append(t)

            # Warmup burst: keeps the PE busy through the first DMA ramp and
            # brings HAM to full clock before the real stream starts.
            wps = psump.tile([SUB_B, 2 * SC], F32, tag="ps", name="wps")
            for _ in range(warmup_mms):
                nc.tensor.matmul(
                    wps[:, 0:SC], wt0[:, 0:SUB_B], wt1[:], start=True, stop=True
                )

            out_r = out.rearrange("(n j p) c -> n p j c", p=SUB_B, j=subs_per_dma)

            for di in range(n_dma):
                xt0 = xtp0.tile([SUB_B, dma_b], BF16, tag="xt0")
                xt1 = xtp1.tile([SUB_B, dma_b], BF16, tag="xt1")
                oh_t = oh_ts[di % oh_bufs_n]
                c0 = di * dma_b
                # xt transfers split in halves, interleaved, so the first
                # packs of the tile unblock after half the transfer.
                hb = dma_b // 2
                for hh in range(2):
                    o = hh * hb
                    nc.sync.dma_start(
                        xt0[:, o : o + hb], xT[0:SUB_B, c0 + o : c0 + o + hb]
                    )
                    nc.sync.dma_start(
                        xt1[:, o : o + hb],
                        xT[SUB_B : 2 * SUB_B, c0 + o : c0 + o + hb],
                    )
                nc.scalar.dma_start(oh_t[0:S, :], oh[:, c0 : c0 + dma_b])

                outb = outp.tile([SUB_B, subs_per_dma * C], F32, tag="outb")
                mega = megap.tile([SUB_B, subs_per_dma * SC], BF16, tag="mega")

                packs_per_red = red_subs // 2
                for pk in range(packs_per_dma):
                    ps = psump.tile([SUB_B, 2 * SC], F32, tag="ps", name="ps")
                    for h in range(2):
                        jj = pk * 2 + h
                        js = jj * SUB_B
                        lo, hi = h * SC, (h + 1) * SC
                        nc.tensor.matmul(
                            ps[:, lo:hi], xt0[:, js : js + SUB_B], wt0[:],
                            start=True, stop=False,
                        )
                        nc.tensor.matmul(
                            ps[:, lo:hi], xt1[:, js : js + SUB_B], wt1[:],
                            start=False, stop=False,
                        )
                        nc.tensor.matmul(
                            ps[:, lo:hi], oh_t[:, js : js + SUB_B], vpen_t[:],
                            start=False, stop=True,
                        )
                    if pk % 2 == 1:
                        nc.vector.tensor_copy(
                            mega[:, pk * 2 * SC : (pk + 1) * 2 * SC], ps[:]
                        )
                    else:
                        nc.scalar.copy(
                            mega[:, pk * 2 * SC : (pk + 1) * 2 * SC], ps[:]
                        )

                    if (pk + 1) % packs_per_red == 0:
                        r = pk // packs_per_red
                        m0 = r * red_subs * C
                        m1 = (r + 1) * red_subs * C
                        nc.vector.tensor_reduce(
                            out=outb[:, m0:m1],
                            in_=mega[:, m0 * S : m1 * S].rearrange(
                                "p (m s) -> p m s", m=m1 - m0, s=S
                            ),
                            axis=mybir.AxisListType.X,
                            op=mybir.AluOpType.max,
                        )
                nc.gpsimd.dma_start(
                    out_r[di],
                    outb[:].rearrange("p (j c) -> p j c", j=subs_per_dma, c=C),
                )

    nc.compile()
    return nc


def _round_bf16(a: np.ndarray) -> np.ndarray:
    """fp32 -> bf16 with round-to-nearest-even, returned as ml_dtypes.bfloat16."""
    import ml_dtypes

    bits = np.ascontiguousarray(a, dtype=np.float32).view(np.uint32)
    lsb = (bits >> np.uint32(16)) & np.uint32(1)
    rounded = ((bits + np.uint32(0x7FFF) + lsb) >> np.uint32(16)).astype(np.uint16)
    return rounded.view(ml_dtypes.bfloat16)


def _host_prep(W, b):
    """Weight-stack layout prep shared by all cores."""
    W = np.asarray(W, dtype=np.float32)
    b = np.asarray(b, dtype=np.float32)
    wt = _round_bf16(np.transpose(W, (2, 1, 0)).reshape(D, SC))
    vpen = np.full((S, SC), -1e30, dtype=np.float32)
    for k in range(S):
        vpen[k, np.arange(C) * S + k] = b[k]
    vpen = _round_bf16(vpen)
    return wt, vpen


_NC_CACHE = {}


def kernel(x, system_id, W, b):
    x = np.asarray(x, dtype=np.float32)
    system_id = np.asarray(system_id)

    key = (x.shape[0],)
    if key not in _NC_CACHE:
        _NC_CACHE[key] = build_nc(x.shape[0] // N_CORES)
    nc = _NC_CACHE[key]

    wt, vpen = _host_prep(W, b)

    n_rows = x.shape[0] // N_CORES
    x_bf = _round_bf16(x)
    eye = np.eye(S, dtype=np.float32)
    in_maps = []
    for core in range(N_CORES):
        lo, hi = core * n_rows, (core + 1) * n_rows
        in_maps.append(
            {
                "xT": np.ascontiguousarray(x_bf[lo:hi].T),           # [D, n_rows]
                "oh": np.ascontiguousarray(
                    _round_bf16(eye[:, system_id[lo:hi]])
                ),                                                    # [S, n_rows]
                "wt": wt,
                "vpen": vpen,
            }
        )

    res = run_bass_kernel_spmd(nc, in_maps, core_ids=list(range(N_CORES)))
    out = np.concatenate([res.results[i]["out"] for i in range(N_CORES)], axis=0)
    return out.astype(np.float32)
